# revision 48
# baseline (speedup 1.0000x reference)
"""Trainium2 Bass kernel for nn_ComplexityDecoderLayer (moe_routing).

Strategy (8 NeuronCores, SPMD, 3 collective-free device launches):
  Launch A1: qkv projections (+mu-guidance) + qk-norm + rope + causal
    attention, head-parallel (2 q-heads per core, kv-head duplicated per
    pair). Outputs RAW softmax numerator pv (fp32) and denominator se;
    softmax normalization happens on host in fp32 (exact), which removes
    all on-device reciprocals/broadcasts AND the AllToAll + its barrier.
  Host glue: normalize attn, redistribute head-parallel -> token-parallel.
  Launch A2: o-proj, mu = attnO @ Wdyn, controller, INL dynamics on the
    core's 256-token slice.
  Host glue: router top-1 (fp64, with exact recompute of near-tie tokens),
    ln2 rmsnorm, token gather by expert.
  Launch B: MoE expert FFN, expert-parallel (2 cores per expert,
    intermediate dim split), capacity factor 1.0 (512 tokens/expert);
    overflow tokens beyond 512 are computed exactly on host fp32.

Dtypes: fp16 matmul operands (fp32 PSUM accumulation), bf16 attention
probabilities (no max-subtraction; bf16 has the range), fp32 elementwise.
"""

import numpy as np

import concourse.bass as bass
import concourse.bacc as bacc
import concourse.tile as tile
from concourse import mybir
from concourse.bass_utils import run_bass_kernel_spmd

# ---- problem constants (hardcoded per spec) ----
T, D, H, KH, HD, I, E, V = 2048, 2048, 16, 4, 128, 5632, 4, 32000
CH, DTC, EPS = 64, 0.1, 1e-6
SCALE = HD ** -0.5
ROPE_BASE = 10000.0
NC = 8                      # cores
TSL = T // NC               # 256 tokens per core
QC = H * HD // NC           # 256 q-cols per core (2 heads)
IH = I // 2                 # 2816 intermediate half per core
CAP = 512                   # MoE per-expert device capacity (factor 1.0)
KS1 = (2 * D) // 128        # 32 k-subtiles for stacked [x; mu_prev] projections
KSD = D // 128              # 16 k-subtiles over D
KSI = IH // 128             # 22 k-subtiles over IH

F32 = mybir.dt.float32
F16 = mybir.dt.float16
BF16 = mybir.dt.bfloat16
AF = mybir.ActivationFunctionType
ALU = mybir.AluOpType

TRACE = False           # set by test.py for profiling
LAST_EXEC_NS = {}       # launch name -> exec ns (when TRACE)

_cache = {}


# ----------------------------------------------------------------- launch A1
def _build_launch_a1():
    nc = bacc.Bacc("TRN2", target_bir_lowering=False, debug=False, num_devices=NC)

    def din(name, shape, dt):
        return nc.dram_tensor(name, shape, dt, kind="ExternalInput")

    # xm re-laid on host: row (nch*2+kg)*128+p holds [ks_local, t] contiguous
    xm = din("xm", [(T // 512) * 2 * 128, 16 * 512], F16)
    # wst re-laid on host: row p holds [ks, m] contiguous (32 KB / partition)
    wst = din("wst", [128, KS1 * (QC + 2 * HD)], F16)
    qn = din("qn", [HD, 1], F32)
    kn = din("kn", [HD, 1], F32)
    qnsw = din("qnsw", [HD, 1], F32)
    knsw = din("knsw", [HD, 1], F32)
    cosf = din("cosf", [HD, T], F32)         # cos rows duplicated for both halves
    sins = din("sins", [HD, T], F32)         # [-sin ; +sin]
    trimask = din("trimask", [128, 128], BF16)

    pvt = nc.dram_tensor("pvt", [2 * HD, T], F32, kind="ExternalOutput")
    set_ = nc.dram_tensor("set", [2, T], F32, kind="ExternalOutput")

    xm_t = xm.ap().rearrange("(nk p) (ks t) -> nk p ks t", p=128, t=512)
    wst_t = wst.ap().rearrange("p (ks m) -> p ks m", m=QC + 2 * HD)

    with tile.TileContext(nc) as tc:
        with tc.tile_pool(name="const", bufs=1) as const:
            qn_sb = const.tile([HD, 1], F32)
            nc.sync.dma_start(out=qn_sb[:], in_=qn[:])
            kn_sb = const.tile([HD, 1], F32)
            nc.sync.dma_start(out=kn_sb[:], in_=kn[:])
            qnsw_sb = const.tile([HD, 1], F32)
            nc.sync.dma_start(out=qnsw_sb[:], in_=qnsw[:])
            knsw_sb = const.tile([HD, 1], F32)
            nc.sync.dma_start(out=knsw_sb[:], in_=knsw[:])
            tri_sb = const.tile([128, 128], BF16)
            nc.sync.dma_start(out=tri_sb[:], in_=trimask[:])
            ones_sb = const.tile([128, 1], BF16)   # sumexp lhsT
            nc.vector.memset(ones_sb[:], 1.0)
            ones16 = const.tile([128, 128], F16)   # ssq broadcast lhsT
            nc.vector.memset(ones16[:], 1.0)
            eps128 = const.tile([128, 1], F32)
            nc.vector.memset(eps128[:], float(EPS))
            cos_sb = const.tile([HD, T], F32)
            nc.gpsimd.dma_start(out=cos_sb[:], in_=cosf[:])
            sin_sb = const.tile([HD, T], F32)
            nc.gpsimd.dma_start(out=sin_sb[:], in_=sins[:])

            with (
                tc.tile_pool(name="acts", bufs=1) as acts,
                tc.tile_pool(name="wp", bufs=1) as wp,
                tc.tile_pool(name="xp", bufs=2) as xp,
                tc.tile_pool(name="ep", bufs=6) as ep,
                tc.tile_pool(name="stg", bufs=3) as stg,
                tc.tile_pool(name="psA", bufs=1, space="PSUM") as psA,
            ):
                qr = [acts.tile([128, T], F16, name=f"qr{h}") for h in range(2)]
                kr = acts.tile([128, T], F16)
                vtok = acts.tile([128, KSD, 128], BF16)
                vraw = acts.tile([128, T], BF16)
                idn = acts.tile([128, 128], BF16)
                nc.gpsimd.memset(idn[:], 0.0)
                from concourse.masks import make_identity
                make_identity(nc, idn[:], nomemset=True)

                wall = wp.tile([128, KS1, QC + 2 * HD], F16)
                for kg in range(4):  # scalar queue: parallel with xch on sync
                    nc.scalar.dma_start(out=wall[:, kg * 8:(kg + 1) * 8, :],
                                        in_=wst_t[:, kg * 8:(kg + 1) * 8, :])

                # ------- phase 1: projections + qk-norm + rope ----
                for nch in range(T // 512):
                    ts = slice(nch * 512, nch * 512 + 512)
                    xch = xp.tile([128, KS1, 512], F16, tag="xch")
                    for kg in range(2):
                        nc.sync.dma_start(
                            out=xch[:, kg * 16:(kg + 1) * 16, :],
                            in_=xm_t[nch * 2 + kg])
                    for gi, moff in enumerate((0, 128, 256, 384)):
                        ps = psA.tile([128, 512], F32, tag="pg", bufs=2)
                        for ks in range(KS1):
                            nc.tensor.matmul(
                                ps[:], wall[:, ks, moff:moff + 128],
                                xch[:, ks, :],
                                start=(ks == 0), stop=(ks == KS1 - 1),
                            )
                        if gi < 3:
                            sq = xp.tile([128, 512], F16, tag="sq")
                            nc.scalar.activation(sq[:], ps[:], AF.Square)
                            # all-ones [128,128] lhsT: ssq lands broadcast
                            # across 128 partitions -> rsqrt is 128-parallel
                            pbr = psA.tile([128, 512], F32, tag="sc", bufs=2)
                            nc.tensor.matmul(pbr[:], ones16[:], sq[:],
                                             start=True, stop=True)
                            rqs = xp.tile([128, 512], F32, tag="rqs")
                            nc.scalar.activation(rqs[:], pbr[:], AF.Sqrt,
                                                 bias=eps128[:], scale=1.0 / HD)
                            rq = xp.tile([128, 512], F32, tag="rq")
                            nc.vector.reciprocal_approx_fast(rq[:], rqs[:])
                            nw = qn_sb if gi < 2 else kn_sb
                            nwsw = qnsw_sb if gi < 2 else knsw_sb
                            dst = qr[gi] if gi < 2 else kr
                            praw = xp.tile([128, 512], F32, tag="praw")
                            nc.vector.tensor_copy(praw[:], ps[:])
                            psw = xp.tile([128, 512], F32, tag="psw")
                            nc.gpsimd.dma_start(out=psw[0:64, :],
                                                in_=praw[64:128, :])
                            nc.gpsimd.dma_start(out=psw[64:128, :],
                                                in_=praw[0:64, :])
                            a_t = xp.tile([128, 512], F32, tag="ra")
                            nc.vector.scalar_tensor_tensor(
                                a_t[:], praw[:], nw[:], cos_sb[:, ts],
                                op0=ALU.mult, op1=ALU.mult)
                            b_t = xp.tile([128, 512], F32, tag="rb")
                            nc.vector.scalar_tensor_tensor(
                                b_t[:], psw[:], nwsw[:], sin_sb[:, ts],
                                op0=ALU.mult, op1=ALU.mult)
                            rr = xp.tile([128, 512], F32, tag="rr")
                            nc.vector.tensor_add(rr[:], a_t[:], b_t[:])
                            nc.vector.tensor_mul(dst[:, ts], rr[:], rq[:])
                        else:
                            nc.vector.tensor_copy(vraw[:, ts], ps[:])
                            for st in range(4 * nch, 4 * nch + 4):
                                ptr = psA.tile([128, 128], BF16, tag="sc",
                                               bufs=2)
                                nc.tensor.transpose(
                                    ptr[:],
                                    vraw[:, st * 128:(st + 1) * 128],
                                    idn[:])
                                nc.vector.tensor_copy(vtok[:, st, :], ptr[:])

                # ------- phase 2: attention (raw pv + sumexp out) ----
                for h in range(2):
                    for tch in range(T // 512):
                        t0 = tch * 512
                        ts = slice(t0, t0 + 512)
                        nsi = min(4 * tch + 4, KSD)
                        pv = psA.tile([128, 512], F32, tag="pv", bufs=2)
                        se = psA.tile([1, 512], F32, tag="paux", bufs=2)
                        for si in range(nsi):
                            off = max(si * 128 - t0, 0)
                            sc = psA.tile([128, 512], F32, tag="sc", bufs=2)
                            nc.tensor.matmul(
                                sc[:, off:512],
                                kr[:, si * 128:(si + 1) * 128],
                                qr[h][:, t0 + off:t0 + 512],
                                start=True, stop=True)
                            eb = ep.tile([128, 512], BF16, tag="eb")
                            if si * 128 >= t0:
                                ebd = ep.tile([128, 128], BF16, tag="ebd")
                                nc.scalar.activation(
                                    ebd[:], sc[:, off:off + 128],
                                    AF.Exp, scale=float(SCALE))
                                nc.vector.tensor_mul(
                                    eb[:, off:off + 128], ebd[:], tri_sb[:])
                                if off + 128 < 512:
                                    nc.scalar.activation(
                                        eb[:, off + 128:512],
                                        sc[:, off + 128:512],
                                        AF.Exp, scale=float(SCALE))
                            else:
                                nc.scalar.activation(
                                    eb[:, off:512], sc[:, off:512],
                                    AF.Exp, scale=float(SCALE))
                            nc.tensor.matmul(pv[:, off:512], vtok[:, si, :],
                                             eb[:, off:512],
                                             start=(si == 0),
                                             stop=(si == nsi - 1))
                            nc.tensor.matmul(se[:, off:512], ones_sb[:],
                                             eb[:, off:512],
                                             start=(si == 0),
                                             stop=(si == nsi - 1))
                        pvs = stg.tile([128, 512], F32, tag="pvs")
                        nc.vector.tensor_copy(pvs[:], pv[:])
                        nc.sync.dma_start(out=pvt[h * 128:(h + 1) * 128, ts],
                                          in_=pvs[:])
                        ses = stg.tile([1, 512], F32, tag="ses")
                        nc.vector.tensor_copy(ses[:], se[:])
                        nc.sync.dma_start(out=set_[h:h + 1, ts], in_=ses[:])

    nc.compile()
    return nc


# ----------------------------------------------------------------- launch A2
def _build_launch_a2():
    """One fused GEMM: [attnO^T ; mu^T] = [Wo | Wo@Wdyn]^T @ agk.

    mu = (attn@Wo)@Wdyn = attn@(Wo@Wdyn); the product weight is folded on
    host in fp64, so mu no longer serializes behind the o-projection.
    Controller + INL dynamics are elementwise + tiny GEMMs -> host fp32.
    """
    nc = bacc.Bacc("TRN2", target_bir_lowering=False, debug=False, num_devices=NC)

    agk = nc.dram_tensor("agk", [128, KSD * TSL], F16, kind="ExternalInput")
    wof = nc.dram_tensor("wof", [2 * D, D], F16, kind="ExternalInput")
    ot = nc.dram_tensor("ot", [2 * D, TSL], F32, kind="ExternalOutput")

    agk_t = agk.ap().rearrange("p (ks t) -> p ks t", t=TSL)
    wof_t = wof.ap().rearrange("(mt p) (ks m) -> mt p ks m", p=128, m=128)

    with tile.TileContext(nc) as tc:
        with (
            tc.tile_pool(name="agp", bufs=1) as agp,
            tc.tile_pool(name="w3", bufs=4) as w3,
            tc.tile_pool(name="st", bufs=4) as st,
            tc.tile_pool(name="ps3", bufs=2, space="PSUM") as ps3,
        ):
            agk_sb = agp.tile([128, KSD, TSL], F16)
            for kq in range(4):  # split: first matmuls start after 1/4
                nc.sync.dma_start(out=agk_sb[:, kq * 4:(kq + 1) * 4, :],
                                  in_=agk_t[:, kq * 4:(kq + 1) * 4, :])

            for mt in range(2 * KSD):
                wom = w3.tile([128, KSD, 128], F16, tag="w")
                nc.sync.dma_start(out=wom[:], in_=wof_t[mt])
                po = ps3.tile([128, TSL], F32, tag="po")
                for ks in range(KSD):
                    nc.tensor.matmul(po[:], wom[:, ks, :], agk_sb[:, ks, :],
                                     start=(ks == 0), stop=(ks == KSD - 1))
                osb = st.tile([128, TSL], F32, tag="osb")
                nc.vector.tensor_copy(osb[:], po[:])
                nc.gpsimd.dma_start(out=ot[mt * 128:(mt + 1) * 128, :],
                                    in_=osb[:])

    nc.compile()
    return nc


# ------------------------------------------------------------------ launch B
def _build_launch_b():
    nc = bacc.Bacc("TRN2", target_bir_lowering=False, debug=False, num_devices=NC)

    # all host-re-laid for contiguous per-partition DMA runs
    x2g = nc.dram_tensor("x2g", [128, KSD * CAP], F16, kind="ExternalInput")
    wg = nc.dram_tensor("wg", [IH, D], F16, kind="ExternalInput")
    wu = nc.dram_tensor("wu", [IH, D], F16, kind="ExternalInput")
    wd = nc.dram_tensor("wd", [D, IH], F16, kind="ExternalInput")
    het = nc.dram_tensor("het", [D, CAP], F32, kind="ExternalOutput")

    x2_t = x2g.ap().rearrange("p (ks t) -> p ks t", t=CAP)
    wg_t = wg.ap().rearrange("(mt p) (ks m) -> mt p ks m", p=128, m=128)
    wu_t = wu.ap().rearrange("(mt p) (ks m) -> mt p ks m", p=128, m=128)
    wd_t = wd.ap().rearrange("(mt p) (ks m) -> mt p ks m", p=128, m=128)

    with tile.TileContext(nc) as tc:
        with (
            tc.tile_pool(name="xc", bufs=1) as xc,
            tc.tile_pool(name="wp", bufs=3) as wp,
            tc.tile_pool(name="ac", bufs=1) as ac,
            tc.tile_pool(name="st", bufs=3) as st,
            tc.tile_pool(name="ps", bufs=2, space="PSUM") as ps,
        ):
            x2sb = xc.tile([128, KSD, CAP], F16)
            for kq in range(4):  # split: first matmuls start after 1/4
                nc.gpsimd.dma_start(out=x2sb[:, kq * 4:(kq + 1) * 4, :],
                                    in_=x2_t[:, kq * 4:(kq + 1) * 4, :])
            act = ac.tile([128, KSI, CAP], F16)

            for mt in range(KSI):
                wgm = wp.tile([128, KSD, 128], F16, tag="wg")
                nc.sync.dma_start(out=wgm[:], in_=wg_t[mt])
                wum = wp.tile([128, KSD, 128], F16, tag="wu")
                nc.sync.dma_start(out=wum[:], in_=wu_t[mt])
                pg = ps.tile([128, CAP], F32, tag="pg")
                pu = ps.tile([128, CAP], F32, tag="pu")
                for ks in range(KSD):
                    nc.tensor.matmul(pg[:], wgm[:, ks, :], x2sb[:, ks, :],
                                     start=(ks == 0), stop=(ks == KSD - 1))
                for ks in range(KSD):
                    nc.tensor.matmul(pu[:], wum[:, ks, :], x2sb[:, ks, :],
                                     start=(ks == 0), stop=(ks == KSD - 1))
                sg = st.tile([128, CAP], F32, tag="sg")
                nc.scalar.activation(sg[:], pg[:], AF.Silu)
                nc.vector.tensor_mul(act[:, mt, :], sg[:], pu[:])

            for mt in range(KSD):
                wdm = wp.tile([128, KSI, 128], F16, tag="wd")
                nc.sync.dma_start(out=wdm[:], in_=wd_t[mt])
                pd = ps.tile([128, CAP], F32, tag="pd")
                for ks in range(KSI):
                    nc.tensor.matmul(pd[:], wdm[:, ks, :], act[:, ks, :],
                                     start=(ks == 0), stop=(ks == KSI - 1))
                ot = st.tile([128, CAP], F32, tag="ot")
                nc.vector.tensor_copy(ot[:], pd[:])
                nc.gpsimd.dma_start(
                    out=het[mt * 128:(mt + 1) * 128, :], in_=ot[:])

    nc.compile()
    return nc


# ------------------------------------------------------------------ host glue
def _rope_tables(positions):
    inv = 1.0 / (ROPE_BASE ** (np.arange(0, HD, 2, dtype=np.float64) / HD))
    ang = positions.astype(np.float64)[:, None] * inv[None, :]     # [T, 64]
    cosT = np.cos(ang).T.astype(np.float32)                        # [64, T]
    sinT = np.sin(ang).T.astype(np.float32)
    cosf = np.concatenate([cosT, cosT], axis=0)                    # [128, T]
    sins = np.concatenate([-sinT, sinT], axis=0)
    return cosf, sins


LAST_RES = {}


def _tile_ksm(W, dt=np.float16):
    """Re-layout [K, M] so that the [128, K/128, 128] tile for output block
    mt is one contiguous 4KB-per-partition DMA: row mt*128+p holds the
    (ks, m) plane contiguously."""
    K, M = W.shape
    return np.ascontiguousarray(
        W.reshape(K // 128, 128, M // 128, 128)
        .transpose(2, 1, 0, 3).reshape(M, K)).astype(dt)


def _tile_pk(W, dt):
    """Re-layout [K, N] (K = ks*128+p) to [128, (K/128)*N]: partition p
    holds all its ks-rows contiguously."""
    K, N = W.shape
    return np.ascontiguousarray(
        W.reshape(K // 128, 128, N).transpose(1, 0, 2).reshape(128, -1)
    ).astype(dt)


def _run(nc_obj, in_maps, name):
    res = run_bass_kernel_spmd(nc_obj, in_maps, list(range(NC)), trace=TRACE)
    if TRACE:
        LAST_EXEC_NS[name] = res.exec_time_ns
        LAST_RES[name] = res
    return res.results


def _np_softmax(x, axis=-1):
    m = x.max(axis=axis, keepdims=True)
    e = np.exp(x - m)
    return e / e.sum(axis=axis, keepdims=True)


def _np_silu(x):
    return x / (1.0 + np.exp(-x))


def _exact_mu_rows(inputs, risk_idx):
    """Reference-faithful fp64 recompute of mu rows for near-tie tokens."""
    f = np.float64
    hidden = inputs["hidden_states"].astype(f)
    mu_prev = inputs["mu_prev"].astype(f)
    pos = np.asarray(inputs["positions"]).astype(np.int64)
    x = hidden * (1.0 / np.sqrt((hidden ** 2).mean(-1, keepdims=True) + EPS))
    x = x * inputs["ln1_w"].astype(f)[None, :]
    k = x @ inputs["Wk"].astype(f) + mu_prev @ inputs["Wmu_k"].astype(f)
    v = x @ inputs["Wv"].astype(f) + mu_prev @ inputs["Wmu_v"].astype(f)
    k = k.reshape(T, KH, HD)
    v = v.reshape(T, KH, HD)
    k = k * (1.0 / np.sqrt((k ** 2).mean(-1, keepdims=True) + EPS))
    k = k * inputs["kn_w"].astype(f)

    inv = 1.0 / (ROPE_BASE ** (np.arange(0, HD, 2, dtype=f) / HD))
    ang = pos[:, None] * inv[None, :]
    cos, sin = np.cos(ang)[:, None, :], np.sin(ang)[:, None, :]

    def rope(t):
        t1, t2 = t[..., :64], t[..., 64:]
        return np.concatenate([t1 * cos - t2 * sin, t2 * cos + t1 * sin], -1)

    k = rope(k)
    Wq = inputs["Wq"].astype(f)
    Wmq = inputs["Wmu_q"].astype(f)
    Wo = inputs["Wo"].astype(f)
    Wdyn = inputs["Wdyn_mu"].astype(f)
    qn_w = inputs["qn_w"].astype(f)
    mu_rows = np.zeros((len(risk_idx), D), np.float64)
    rep = H // KH
    for n, t in enumerate(risk_idx):
        q = x[t] @ Wq + mu_prev[t] @ Wmq
        q = q.reshape(H, HD)
        q = q * (1.0 / np.sqrt((q ** 2).mean(-1, keepdims=True) + EPS)) * qn_w
        q1, q2 = q[:, :64], q[:, 64:]
        c, s = np.cos(ang[t]), np.sin(ang[t])
        q = np.concatenate([q1 * c - q2 * s, q2 * c + q1 * s], -1)    # [H, HD]
        kk = np.repeat(k[: t + 1], rep, axis=1)                      # [t+1, H, HD]
        vv = np.repeat(v[: t + 1], rep, axis=1)
        sc = np.einsum("hd,shd->hs", q, kk) * SCALE
        pr = _np_softmax(sc, axis=-1)
        at = np.einsum("hs,shd->hd", pr, vv).reshape(H * HD)
        mu_rows[n] = (at @ Wo) @ Wdyn
    return mu_rows


def kernel(**inputs):
    f32 = np.float32
    hidden = np.ascontiguousarray(inputs["hidden_states"], dtype=f32)
    vel = np.ascontiguousarray(inputs["velocity_states"], dtype=f32)
    mu_prev = np.ascontiguousarray(inputs["mu_prev"], dtype=f32)
    pos = np.asarray(inputs["positions"]).astype(np.int64)
    tok = np.asarray(inputs["token_ids"]).astype(np.int64)
    ln1 = np.asarray(inputs["ln1_w"], dtype=f32)
    ln2 = np.asarray(inputs["ln2_w"], dtype=f32)

    # ---- host prep for launch A1 ----
    r1 = 1.0 / np.sqrt((hidden.astype(np.float64) ** 2).mean(-1) + EPS)
    xT = (hidden * r1[:, None].astype(f32) * ln1[None, :]).T
    xmT = np.concatenate([xT, mu_prev.T], axis=0).astype(np.float16)  # [2D, T]
    # re-layout: row (nch*2+kg)*128+p holds that chunk's [ks_local, t] plane
    xmh = np.ascontiguousarray(
        xmT.reshape(2, 16, 128, 4, 512).transpose(3, 0, 2, 1, 4)
        .reshape(1024, 8192))

    def colcat(*ws):
        return np.concatenate(ws, axis=1)

    cosf, sins = _rope_tables(pos)
    trimask = np.triu(np.ones((128, 128), np.float32))  # [s, t]: t >= s
    import ml_dtypes
    trimask = trimask.astype(ml_dtypes.bfloat16)

    Wq = np.asarray(inputs["Wq"], f32)
    Wmq = np.asarray(inputs["Wmu_q"], f32)
    Wk = np.asarray(inputs["Wk"], f32)
    Wmk = np.asarray(inputs["Wmu_k"], f32)
    Wv = np.asarray(inputs["Wv"], f32)
    Wmv = np.asarray(inputs["Wmu_v"], f32)
    qn = np.asarray(inputs["qn_w"], f32).reshape(HD, 1)
    kn = np.asarray(inputs["kn_w"], f32).reshape(HD, 1)

    if "A1" not in _cache:
        _cache["A1"] = _build_launch_a1()
    in_maps = []
    for c in range(NC):
        g = c // 2
        qs = slice(c * QC, (c + 1) * QC)
        ks_ = slice(g * HD, (g + 1) * HD)
        wst = _tile_pk(colcat(
            np.concatenate([Wq[:, qs], Wmq[:, qs]], axis=0),
            np.concatenate([Wk[:, ks_], Wmk[:, ks_]], axis=0),
            np.concatenate([Wv[:, ks_], Wmv[:, ks_]], axis=0),
        ), np.float16)  # [128, 32*512]
        in_maps.append({
            "xm": xmh, "wst": wst,
            "qn": qn, "kn": kn,
            "qnsw": np.concatenate([qn[64:], qn[:64]], axis=0),
            "knsw": np.concatenate([kn[64:], kn[:64]], axis=0),
            "cosf": cosf, "sins": sins,
            "trimask": trimask,
        })
    res_a1 = _run(_cache["A1"], in_maps, "A1")

    # ---- host: softmax normalization + head->token redistribution ----
    pv_all = np.concatenate([res_a1[c]["pvt"] for c in range(NC)], axis=0)
    se_all = np.concatenate([res_a1[c]["set"] for c in range(NC)], axis=0)
    attnT = pv_all / np.repeat(se_all, HD, axis=0)          # [H*HD, T] fp32
    agkT = attnT.astype(np.float16)

    # ---- launch A2: fused [o-proj | mu] GEMM ----
    if "A2" not in _cache:
        _cache["A2"] = _build_launch_a2()
    Wo64 = np.asarray(inputs["Wo"], np.float64)
    Wcat = np.concatenate(
        [Wo64, Wo64 @ np.asarray(inputs["Wdyn_mu"], np.float64)],
        axis=1).astype(f32)                                    # [D, 2D]
    wof = _tile_ksm(Wcat)
    in_maps_a2 = []
    for c in range(NC):
        tsl = slice(c * TSL, (c + 1) * TSL)
        in_maps_a2.append({
            "agk": _tile_pk(agkT[:, tsl], np.float16),
            "wof": wof,
        })
    res_a2 = _run(_cache["A2"], in_maps_a2, "A2")

    omu = np.concatenate([res_a2[c]["ot"] for c in range(NC)], axis=1)
    attn_o = np.ascontiguousarray(omu[:D].T, f32)              # [T, D]
    mu = np.ascontiguousarray(omu[D:].T, f32)

    # ---- controller + INL dynamics on host (fp32, exact) ----
    Wc1f = np.asarray(inputs["Wc1"], f32)
    Wc2f = np.asarray(inputs["Wc2"], f32)
    bc1f = np.asarray(inputs["bc1"], f32)
    bc2f = np.asarray(inputs["bc2"], f32)
    ctrl = 1.0 / (1.0 + np.exp(-(np.tanh(attn_o @ Wc1f + bc1f) @ Wc2f + bc2f)))
    alpha, beta = ctrl[:, :D], ctrl[:, D:]
    v_new = vel + np.float32(DTC) * (alpha * (mu - attn_o) - beta * vel)
    v_new = v_new.astype(f32)
    hidden2 = (hidden + attn_o + np.float32(DTC) * v_new).astype(f32)

    # ---- routing on host (fp64; near-tie tokens recomputed exactly) ----
    rt = np.asarray(inputs["router_table"], f32)[tok]              # [T, E]
    Wmur = np.asarray(inputs["Wmu_router"], f32)
    logits = rt.astype(np.float64) + mu.astype(np.float64) @ Wmur.astype(np.float64)
    srt = np.sort(logits, axis=-1)
    risk = np.where(srt[:, -1] - srt[:, -2] < 2e-3)[0]
    if len(risk) > 0:
        mu_fix = _exact_mu_rows(inputs, risk)
        logits[risk] = rt[risk].astype(np.float64) + mu_fix @ Wmur.astype(np.float64)
    eidx = logits.argmax(-1)
    gate = _np_softmax(logits, axis=-1)[np.arange(T), eidx].astype(f32)

    # ---- launch B: gathered expert FFN (capacity 512 + host overflow) ----
    r2 = 1.0 / np.sqrt((hidden2.astype(np.float64) ** 2).mean(-1) + EPS)
    x2 = hidden2 * r2[:, None].astype(f32) * ln2[None, :]
    Wg = np.asarray(inputs["Wg"], f32)
    Wu = np.asarray(inputs["Wu"], f32)
    Wd = np.asarray(inputs["Wd"], f32)

    idx_e = [np.where(eidx == e)[0] for e in range(E)]
    dev_counts = [min(len(ix), CAP) for ix in idx_e]
    if "B" not in _cache:
        _cache["B"] = _build_launch_b()

    in_maps_b = []
    for c in range(NC):
        e, m = c // 2, c % 2
        hs = slice(m * IH, (m + 1) * IH)
        x2gT = np.zeros((D, CAP), np.float16)
        x2gT[:, :dev_counts[e]] = x2[idx_e[e][:dev_counts[e]]].T
        in_maps_b.append({
            "x2g": _tile_pk(x2gT, np.float16),
            "wg": _tile_ksm(Wg[e][:, hs]),
            "wu": _tile_ksm(Wu[e][:, hs]),
            "wd": _tile_ksm(Wd[e][hs, :]),
        })
    res_b = _run(_cache["B"], in_maps_b, "B")

    out = hidden2.copy()
    for e in range(E):
        n = dev_counts[e]
        if n > 0:
            he = (res_b[2 * e]["het"][:, :n] + res_b[2 * e + 1]["het"][:, :n]).T
            ix = idx_e[e][:n]
            out[ix] += gate[ix, None] * he
        ov = idx_e[e][n:]
        if len(ov) > 0:  # capacity overflow: exact host fp32 FFN
            xo = x2[ov]
            he_ov = (_np_silu(xo @ Wg[e]) * (xo @ Wu[e])) @ Wd[e]
            out[ov] += gate[ov, None] * he_ov

    return out, v_new, mu
